# revision 6
# baseline (speedup 1.0000x reference)
"""Multi-head attention (B=4, T=2048, D=1024, H=16) on 8 TRN2 NeuronCores.

Sharding: core c handles batch b = c//2 and head-half hh = c%2 (8 heads,
512 of the 1024 channel dims). Each core computes its half of the head
outputs and a row-sharded output projection, producing a partial
[T, D] output. Host unshard: out[b] = partial[2b] + partial[2b+1]
+ b_o + b_v @ w_o.T (the value-bias contribution commutes through
attention because softmax rows sum to 1).

All matmul operands are bf16 (hosts converts inputs); PSUM accumulates
fp32. Score PSUM is double-buffered ([128, 2x512] tiles) so the PE
computes the next group's scores while ACT runs exp on the previous
one, keeping the PE dense enough to hold its max p-state.
"""

from contextlib import ExitStack

import ml_dtypes
import numpy as np

import concourse.bass as bass
import concourse.mybir as mybir
import concourse.tile as tile
from concourse import bacc
from concourse.bass_utils import run_bass_kernel_spmd

B, T, D = 4, 2048, 1024
H = 16
DH = 64  # head dim
HALF = 512  # channels per core (8 heads)
N_CORES = 8

F32 = mybir.dt.float32
BF16 = mybir.dt.bfloat16
NPBF16 = ml_dtypes.bfloat16

TB = 512  # t-block for moving operands
NTB = T // TB  # 4
KB = 128  # contraction block
NKB = D // KB  # 8
NJB = HALF // KB  # 4 j-blocks of the half
NTK = T // KB  # 16 tk blocks


def build_kernel():
    nc = bacc.Bacc(
        "TRN2", target_bir_lowering=False, debug=False, num_devices=N_CORES
    )
    xqT = nc.dram_tensor("xqT", [D, T], BF16, kind="ExternalInput").ap()
    xkT = nc.dram_tensor("xkT", [D, T], BF16, kind="ExternalInput").ap()
    xvT = nc.dram_tensor("xvT", [D, T], BF16, kind="ExternalInput").ap()
    wqT = nc.dram_tensor("wqT", [D, HALF], BF16, kind="ExternalInput").ap()
    wkT = nc.dram_tensor("wkT", [D, HALF], BF16, kind="ExternalInput").ap()
    wvT = nc.dram_tensor("wvT", [D, HALF], BF16, kind="ExternalInput").ap()
    woT = nc.dram_tensor("woT", [HALF, D], BF16, kind="ExternalInput").ap()
    bq = nc.dram_tensor("bq", [HALF, 1], F32, kind="ExternalInput").ap()
    bk = nc.dram_tensor("bk", [HALF, 1], F32, kind="ExternalInput").ap()
    partial = nc.dram_tensor("partial", [T, D], F32, kind="ExternalOutput").ap()

    with tile.TileContext(nc) as tc, ExitStack() as ctx:
        p_const = ctx.enter_context(tc.tile_pool(name="const", bufs=1))
        p_kt = ctx.enter_context(tc.tile_pool(name="kt", bufs=NJB))
        p_qt = ctx.enter_context(tc.tile_pool(name="qt", bufs=NJB))
        p_v = ctx.enter_context(tc.tile_pool(name="v", bufs=NTK))
        p_xs = ctx.enter_context(tc.tile_pool(name="xs", bufs=8))
        p_ex = ctx.enter_context(tc.tile_pool(name="ex", bufs=6))
        p_ot = ctx.enter_context(tc.tile_pool(name="ot", bufs=8))
        p_rc = ctx.enter_context(tc.tile_pool(name="rc", bufs=6))
        p_st = ctx.enter_context(tc.tile_pool(name="st", bufs=2))
        # PSUM: scores 2x2 banks + av 2x1 + proj 2x1 = 8 banks
        p_sc = ctx.enter_context(tc.tile_pool(name="sc", bufs=2, space="PSUM"))
        p_av = ctx.enter_context(tc.tile_pool(name="av", bufs=2, space="PSUM"))
        p_pj = ctx.enter_context(tc.tile_pool(name="pj", bufs=2, space="PSUM"))

        # ---- constants ----
        w_q = p_const.tile([KB, NKB, HALF], BF16, tag="wq")
        nc.sync.dma_start(w_q[:], wqT.rearrange("(kb p) j -> p kb j", p=KB))
        w_k = p_const.tile([KB, NKB, HALF], BF16, tag="wk")
        nc.sync.dma_start(w_k[:], wkT.rearrange("(kb p) j -> p kb j", p=KB))
        w_v = p_const.tile([KB, NKB, HALF], BF16, tag="wv")
        nc.sync.dma_start(w_v[:], wvT.rearrange("(kb p) j -> p kb j", p=KB))
        w_o = p_const.tile([KB, NJB, D], BF16, tag="wo")
        nc.sync.dma_start(w_o[:], woT.rearrange("(jb p) n -> p jb n", p=KB))
        b_q = p_const.tile([KB, NJB], F32, tag="bq")
        nc.sync.dma_start(b_q[:], bq.rearrange("(jb p) one -> p (jb one)", p=KB))
        b_k = p_const.tile([KB, NJB], F32, tag="bk")
        nc.sync.dma_start(b_k[:], bk.rearrange("(jb p) one -> p (jb one)", p=KB))

        # ---- K^T / Q^T projections: {kt,qt}[jb] is [128 (j), T] bf16 ----
        kt_tiles = [p_kt.tile([KB, T], BF16, tag="kt", name=f"kt{j}") for j in range(NJB)]
        qt_tiles = [p_qt.tile([KB, T], BF16, tag="qt", name=f"qt{j}") for j in range(NJB)]
        for x_in, w_in, b_in, dst in (
            (xkT, w_k, b_k, kt_tiles),
            (xqT, w_q, b_q, qt_tiles),
        ):
            for tb in range(NTB):
                for h in range(2):  # jb pair (2h, 2h+1) per psum unit
                    ps = p_sc.tile([KB, 2, TB], F32, tag="sc", name=f"psp{tb}_{h}")
                    for kb in range(NKB):
                        xt = p_xs.tile([KB, TB], BF16, tag="xs")
                        nc.sync.dma_start(
                            xt[:],
                            x_in[kb * KB : (kb + 1) * KB, tb * TB : (tb + 1) * TB],
                        )
                        for u in range(2):
                            jb = 2 * h + u
                            nc.tensor.matmul(
                                ps[:, u, :],
                                w_in[:, kb, jb * KB : (jb + 1) * KB],
                                xt[:],
                                start=(kb == 0),
                                stop=(kb == NKB - 1),
                            )
                    for u in range(2):
                        jb = 2 * h + u
                        nc.vector.tensor_scalar_add(
                            dst[jb][:, tb * TB : (tb + 1) * TB],
                            ps[:, u, :],
                            b_in[:, jb : jb + 1],
                        )

        # ---- V projection (natural layout): V[tk] is [128 (t), 8 (h), 65] ----
        # column 64 of each head is 1.0: the AV matmul then accumulates the
        # softmax denominator in psum row 64 for free.
        v_tiles = [
            p_v.tile([KB, H // 2, DH + 1], BF16, tag="v", name=f"v{j}")
            for j in range(NTK)
        ]
        for t in range(NTK):
            nc.vector.memset(v_tiles[t][:, :, DH : DH + 1], 1.0)
        for tb in range(NTB):
            for h in range(2):  # ts pair (2h, 2h+1) per psum unit
                ps = p_sc.tile([KB, 2, TB], F32, tag="sc", name=f"psv{tb}_{h}")
                for kb in range(NKB):
                    xt = p_xs.tile([KB, TB], BF16, tag="xs")
                    nc.sync.dma_start(
                        xt[:], xvT[kb * KB : (kb + 1) * KB, tb * TB : (tb + 1) * TB]
                    )
                    for u in range(2):
                        ts = 2 * h + u
                        nc.tensor.matmul(
                            ps[:, u, :],
                            xt[:, ts * KB : (ts + 1) * KB],
                            w_v[:, kb, :],
                            start=(kb == 0),
                            stop=(kb == NKB - 1),
                        )
                for u in range(2):
                    ts = 2 * h + u
                    nc.vector.tensor_copy(
                        v_tiles[tb * 4 + ts][:, :, 0:DH],
                        ps[:, u, :].rearrange("p (h d) -> p h d", d=DH),
                    )

        # ---- per t-block: attention + out-projection ----
        # The out-projection for t-block tq is emitted interleaved into the
        # score groups of t-block tq+1 so the PE never stalls waiting for the
        # last head-pair's softmax normalization.
        def emit_po_chain(ot_tiles, tq, nb, ts):
            po = p_pj.tile([KB, TB], F32, tag="po", name=f"po{tq}_{nb}_{ts}")
            for jp in range(NJB):
                nc.tensor.matmul(
                    po[:],
                    ot_tiles[jp][:, ts * KB : (ts + 1) * KB],
                    w_o[:, jp, nb * TB : (nb + 1) * TB],
                    start=(jp == 0),
                    stop=(jp == NJB - 1),
                )
            st = p_st.tile([KB, TB], F32, tag="st", name=f"st{tq}_{nb}_{ts}")
            nc.vector.tensor_copy(st[:], po[:])
            nc.sync.dma_start(
                partial[
                    tq * TB + ts * KB : tq * TB + (ts + 1) * KB,
                    nb * TB : (nb + 1) * TB,
                ],
                st[:],
            )

        pending = []  # deferred out-proj chains from the previous t-block
        for tq in range(NTB):
            ot_tiles = [
                p_ot.tile([KB, TB], BF16, tag="ot", name=f"ot{tq}_{j}")
                for j in range(NJB)
            ]
            gctr = 0
            for jp in range(NJB):  # head pair (2*jp, 2*jp+1)
                avs = [
                    p_av.tile([DH + 1, TB], F32, tag="av", name=f"av{i}")
                    for i in range(2)
                ]
                for g in range(NTK):
                    sc = p_sc.tile([KB, 2, TB], F32, tag="sc")
                    for i in range(2):
                        nc.tensor.matmul(
                            sc[:, i, :],
                            kt_tiles[jp][i * DH : (i + 1) * DH, g * KB : (g + 1) * KB],
                            qt_tiles[jp][i * DH : (i + 1) * DH, tq * TB : (tq + 1) * TB],
                            start=True,
                            stop=True,
                        )
                    ex = p_ex.tile([KB, 2, TB], BF16, tag="ex")
                    nc.scalar.activation(
                        ex[:], sc[:], mybir.ActivationFunctionType.Exp, scale=0.125
                    )
                    for i in range(2):
                        nc.tensor.matmul(
                            avs[i][:],
                            v_tiles[g][:, 2 * jp + i, :],
                            ex[:, i, :],
                            start=(g == 0),
                            stop=(g == NTK - 1),
                        )
                    gctr += 1
                for _ in range(2):
                    if pending:
                        emit_po_chain(*pending.pop(0))
                for i in range(2):
                    # copy the whole AV psum (including the denominator row 64)
                    # to SBUF immediately so the psum bank frees for the next
                    # head pair; normalize from the SBUF copy.
                    asb = p_rc.tile([DH + 1, TB], F32, tag="asb")
                    nc.vector.tensor_copy(asb[:], avs[i][:])
                    bc = p_rc.tile([DH, TB], F32, tag="bc")
                    nc.sync.dma_start(
                        bc[:],
                        asb[DH : DH + 1, None, :].broadcast_to([1, DH, TB]),
                    )
                    rc2 = p_rc.tile([DH, TB], F32, tag="rc2")
                    nc.vector.reciprocal_approx_fast(rc2[:], bc[:])
                    if i == 0:
                        nc.vector.tensor_mul(ot_tiles[jp][0:DH, :], asb[0:DH, :], rc2[:])
                    else:
                        # DVE can't shift partitions; stage then DMA into rows 64:128
                        stg = p_rc.tile([DH, TB], BF16, tag="stg")
                        nc.vector.tensor_mul(stg[:], asb[0:DH, :], rc2[:])
                        nc.sync.dma_start(ot_tiles[jp][DH : 2 * DH, :], stg[:])

            pending = [(ot_tiles, tq, nb, ts) for nb in range(2) for ts in range(4)]

        for args in pending:
            emit_po_chain(*args)

    nc.compile()
    return nc


def kernel(**inputs: np.ndarray) -> np.ndarray:
    query = np.asarray(inputs["query"], dtype=np.float32)
    key = np.asarray(inputs["key"], dtype=np.float32)
    value = np.asarray(inputs["value"], dtype=np.float32)
    w_q = np.asarray(inputs["w_q"], dtype=np.float32)
    b_q = np.asarray(inputs["b_q"], dtype=np.float32)
    w_k = np.asarray(inputs["w_k"], dtype=np.float32)
    b_k = np.asarray(inputs["b_k"], dtype=np.float32)
    w_v = np.asarray(inputs["w_v"], dtype=np.float32)
    b_v = np.asarray(inputs["b_v"], dtype=np.float32)
    w_o = np.asarray(inputs["w_o"], dtype=np.float32)
    b_o = np.asarray(inputs["b_o"], dtype=np.float32)

    nc = build_kernel()

    in_maps = []
    for c in range(N_CORES):
        b = c // 2
        hh = c % 2
        sl = slice(hh * HALF, (hh + 1) * HALF)
        in_maps.append(
            {
                "xqT": np.ascontiguousarray(query[b].T).astype(NPBF16),
                "xkT": np.ascontiguousarray(key[b].T).astype(NPBF16),
                "xvT": np.ascontiguousarray(value[b].T).astype(NPBF16),
                "wqT": np.ascontiguousarray(w_q[sl, :].T).astype(NPBF16),
                "wkT": np.ascontiguousarray(w_k[sl, :].T).astype(NPBF16),
                "wvT": np.ascontiguousarray(w_v[sl, :].T).astype(NPBF16),
                "woT": np.ascontiguousarray(w_o[:, sl].T).astype(NPBF16),
                "bq": np.ascontiguousarray(b_q[sl].reshape(HALF, 1)),
                "bk": np.ascontiguousarray(b_k[sl].reshape(HALF, 1)),
            }
        )

    res = run_bass_kernel_spmd(nc, in_maps, core_ids=list(range(N_CORES)))

    const_row = (b_v[None, :] @ w_o.T + b_o[None, :]).astype(np.float32)
    out = np.empty((B, T, D), dtype=np.float32)
    for b in range(B):
        out[b] = res.results[2 * b]["partial"] + res.results[2 * b + 1]["partial"]
        out[b] += const_row
    return out


# revision 7
# speedup vs baseline: 1.1876x; 1.1876x over previous
"""Multi-head attention (B=4, T=2048, D=1024, H=16) on 8 TRN2 NeuronCores.

Sharding: core c handles batch b = c//2 and head-half hh = c%2 (8 heads,
512 of the 1024 channel dims). Each core computes its half of the head
outputs and a row-sharded output projection, producing a partial
[T, D] output. Host unshard: out[b] = partial[2b] + partial[2b+1]
+ b_o + b_v @ w_o.T (the value-bias contribution commutes through
attention because softmax rows sum to 1).

All matmul operands are bf16 (hosts converts inputs); PSUM accumulates
fp32. Score PSUM is double-buffered ([128, 2x512] tiles) so the PE
computes the next group's scores while ACT runs exp on the previous
one, keeping the PE dense enough to hold its max p-state.
"""

from contextlib import ExitStack

import ml_dtypes
import numpy as np

import concourse.bass as bass
import concourse.mybir as mybir
import concourse.tile as tile
from concourse import bacc
from concourse.bass_utils import run_bass_kernel_spmd

B, T, D = 4, 2048, 1024
H = 16
DH = 64  # head dim
HALF = 512  # channels per core (8 heads)
N_CORES = 8

F32 = mybir.dt.float32
BF16 = mybir.dt.bfloat16
NPBF16 = ml_dtypes.bfloat16

TB = 512  # t-block for moving operands
NTB = T // TB  # 4
KB = 128  # contraction block
NKB = D // KB  # 8
NJB = HALF // KB  # 4 j-blocks of the half
NTK = T // KB  # 16 tk blocks


def build_kernel():
    nc = bacc.Bacc(
        "TRN2", target_bir_lowering=False, debug=False, num_devices=N_CORES
    )
    xqT = nc.dram_tensor("xqT", [D, T], BF16, kind="ExternalInput").ap()
    xkT = nc.dram_tensor("xkT", [D, T], BF16, kind="ExternalInput").ap()
    xvT = nc.dram_tensor("xvT", [D, T], BF16, kind="ExternalInput").ap()
    wqT = nc.dram_tensor("wqT", [D, HALF], BF16, kind="ExternalInput").ap()
    wkT = nc.dram_tensor("wkT", [D, HALF], BF16, kind="ExternalInput").ap()
    wvT = nc.dram_tensor("wvT", [D, HALF], BF16, kind="ExternalInput").ap()
    woT = nc.dram_tensor("woT", [HALF, D], BF16, kind="ExternalInput").ap()
    bq = nc.dram_tensor("bq", [HALF, 1], F32, kind="ExternalInput").ap()
    bk = nc.dram_tensor("bk", [HALF, 1], F32, kind="ExternalInput").ap()
    partial = nc.dram_tensor("partial", [T, D], F32, kind="ExternalOutput").ap()

    with tile.TileContext(nc) as tc, ExitStack() as ctx:
        p_const = ctx.enter_context(tc.tile_pool(name="const", bufs=1))
        p_kt = ctx.enter_context(tc.tile_pool(name="kt", bufs=NJB))
        p_qt = ctx.enter_context(tc.tile_pool(name="qt", bufs=NJB))
        p_v = ctx.enter_context(tc.tile_pool(name="v", bufs=NTK))
        p_xs = ctx.enter_context(tc.tile_pool(name="xs", bufs=8))
        p_ex = ctx.enter_context(tc.tile_pool(name="ex", bufs=6))
        p_ot = ctx.enter_context(tc.tile_pool(name="ot", bufs=8))
        p_rc = ctx.enter_context(tc.tile_pool(name="rc", bufs=6))
        p_st = ctx.enter_context(tc.tile_pool(name="st", bufs=2))
        # PSUM: scores 2x2 banks + av 2x1 + proj 2x1 = 8 banks
        p_sc = ctx.enter_context(tc.tile_pool(name="sc", bufs=2, space="PSUM"))
        p_av = ctx.enter_context(tc.tile_pool(name="av", bufs=2, space="PSUM"))
        p_pj = ctx.enter_context(tc.tile_pool(name="pj", bufs=2, space="PSUM"))

        # ---- constants ----
        w_q = p_const.tile([KB, NKB, HALF], BF16, tag="wq")
        nc.sync.dma_start(w_q[:], wqT.rearrange("(kb p) j -> p kb j", p=KB))
        w_k = p_const.tile([KB, NKB, HALF], BF16, tag="wk")
        nc.sync.dma_start(w_k[:], wkT.rearrange("(kb p) j -> p kb j", p=KB))
        w_v = p_const.tile([KB, NKB, HALF], BF16, tag="wv")
        nc.sync.dma_start(w_v[:], wvT.rearrange("(kb p) j -> p kb j", p=KB))
        w_o = p_const.tile([KB, NJB, D], BF16, tag="wo")
        nc.sync.dma_start(w_o[:], woT.rearrange("(jb p) n -> p jb n", p=KB))
        b_q = p_const.tile([KB, NJB], F32, tag="bq")
        nc.sync.dma_start(b_q[:], bq.rearrange("(jb p) one -> p (jb one)", p=KB))
        b_k = p_const.tile([KB, NJB], F32, tag="bk")
        nc.sync.dma_start(b_k[:], bk.rearrange("(jb p) one -> p (jb one)", p=KB))

        # ---- K^T / Q^T projections: {kt,qt}[jb] is [128 (j), T] bf16 ----
        kt_tiles = [p_kt.tile([KB, T], BF16, tag="kt", name=f"kt{j}") for j in range(NJB)]
        qt_tiles = [p_qt.tile([KB, T], BF16, tag="qt", name=f"qt{j}") for j in range(NJB)]
        for x_in, w_in, b_in, dst in (
            (xkT, w_k, b_k, kt_tiles),
            (xqT, w_q, b_q, qt_tiles),
        ):
            for tb in range(NTB):
                # 4 accumulators per tb: one 2-slot sc tile + two 1-bank pj
                # tiles; rings leave a full tb between reuse so the bias-add
                # drains overlap the next tb's matmuls.
                ps = p_sc.tile([KB, 2, TB], F32, tag="sc", name=f"psp{tb}")
                pos = [
                    p_pj.tile([KB, TB], F32, tag="po", name=f"pop{tb}_{u}")
                    for u in range(2)
                ]
                targets = [ps[:, 0, :], ps[:, 1, :], pos[0][:], pos[1][:]]
                for kb in range(NKB):
                    xt = p_xs.tile([KB, TB], BF16, tag="xs")
                    nc.sync.dma_start(
                        xt[:],
                        x_in[kb * KB : (kb + 1) * KB, tb * TB : (tb + 1) * TB],
                    )
                    for jb in range(NJB):
                        nc.tensor.matmul(
                            targets[jb],
                            w_in[:, kb, jb * KB : (jb + 1) * KB],
                            xt[:],
                            start=(kb == 0),
                            stop=(kb == NKB - 1),
                        )
                for jb in range(NJB):
                    nc.vector.tensor_scalar_add(
                        dst[jb][:, tb * TB : (tb + 1) * TB],
                        targets[jb],
                        b_in[:, jb : jb + 1],
                    )

        # ---- V projection (natural layout): V[tk] is [128 (t), 8 (h), 65] ----
        # column 64 of each head is 1.0: the AV matmul then accumulates the
        # softmax denominator in psum row 64 for free.
        v_tiles = [
            p_v.tile([KB, H // 2, DH + 1], BF16, tag="v", name=f"v{j}")
            for j in range(NTK)
        ]
        for t in range(NTK):
            nc.vector.memset(v_tiles[t][:, :, DH : DH + 1], 1.0)
        for tb in range(NTB):
            ps = p_sc.tile([KB, 2, TB], F32, tag="sc", name=f"psv{tb}")
            pos = [
                p_pj.tile([KB, TB], F32, tag="po", name=f"pov{tb}_{u}")
                for u in range(2)
            ]
            targets = [ps[:, 0, :], ps[:, 1, :], pos[0][:], pos[1][:]]
            for kb in range(NKB):
                xt = p_xs.tile([KB, TB], BF16, tag="xs")
                nc.sync.dma_start(
                    xt[:], xvT[kb * KB : (kb + 1) * KB, tb * TB : (tb + 1) * TB]
                )
                for ts in range(4):
                    nc.tensor.matmul(
                        targets[ts],
                        xt[:, ts * KB : (ts + 1) * KB],
                        w_v[:, kb, :],
                        start=(kb == 0),
                        stop=(kb == NKB - 1),
                    )
            for ts in range(4):
                nc.vector.tensor_copy(
                    v_tiles[tb * 4 + ts][:, :, 0:DH],
                    targets[ts].rearrange("p (h d) -> p h d", d=DH),
                )

        # ---- per t-block: attention + out-projection ----
        # The out-projection for t-block tq is emitted interleaved into the
        # score groups of t-block tq+1 so the PE never stalls waiting for the
        # last head-pair's softmax normalization.
        def emit_po_chain(ot_tiles, tq, nb, ts):
            po = p_pj.tile([KB, TB], F32, tag="po", name=f"po{tq}_{nb}_{ts}")
            for jp in range(NJB):
                nc.tensor.matmul(
                    po[:],
                    ot_tiles[jp][:, ts * KB : (ts + 1) * KB],
                    w_o[:, jp, nb * TB : (nb + 1) * TB],
                    start=(jp == 0),
                    stop=(jp == NJB - 1),
                )
            st = p_st.tile([KB, TB], F32, tag="st", name=f"st{tq}_{nb}_{ts}")
            nc.vector.tensor_copy(st[:], po[:])
            nc.sync.dma_start(
                partial[
                    tq * TB + ts * KB : tq * TB + (ts + 1) * KB,
                    nb * TB : (nb + 1) * TB,
                ],
                st[:],
            )

        pending = []  # deferred out-proj chains from the previous t-block
        stg_dmas = []  # deferred ot row-64:128 staging DMAs within a t-block
        for tq in range(NTB):
            ot_tiles = [
                p_ot.tile([KB, TB], BF16, tag="ot", name=f"ot{tq}_{j}")
                for j in range(NJB)
            ]
            gctr = 0
            for jp in range(NJB):  # head pair (2*jp, 2*jp+1)
                avs = [
                    p_av.tile([DH + 1, TB], F32, tag="av", name=f"av{i}")
                    for i in range(2)
                ]
                for g in range(NTK):
                    sc = p_sc.tile([KB, 2, TB], F32, tag="sc")
                    for i in range(2):
                        nc.tensor.matmul(
                            sc[:, i, :],
                            kt_tiles[jp][i * DH : (i + 1) * DH, g * KB : (g + 1) * KB],
                            qt_tiles[jp][i * DH : (i + 1) * DH, tq * TB : (tq + 1) * TB],
                            start=True,
                            stop=True,
                        )
                    ex = p_ex.tile([KB, 2, TB], BF16, tag="ex")
                    nc.scalar.activation(
                        ex[:], sc[:], mybir.ActivationFunctionType.Exp, scale=0.125
                    )
                    for i in range(2):
                        nc.tensor.matmul(
                            avs[i][:],
                            v_tiles[g][:, 2 * jp + i, :],
                            ex[:, i, :],
                            start=(g == 0),
                            stop=(g == NTK - 1),
                        )
                    gctr += 1
                for _ in range(2):
                    if pending:
                        emit_po_chain(*pending.pop(0))
                for i in range(2):
                    # copy the whole AV psum (including the denominator row 64)
                    # to SBUF immediately so the psum bank frees for the next
                    # head pair; normalize from the SBUF copy.
                    asb = p_rc.tile([DH + 1, TB], F32, tag="asb")
                    nc.vector.tensor_copy(asb[:], avs[i][:])
                    bc = p_rc.tile([DH, TB], F32, tag="bc")
                    nc.sync.dma_start(
                        bc[:],
                        asb[DH : DH + 1, None, :].broadcast_to([1, DH, TB]),
                    )
                    rc2 = p_rc.tile([DH, TB], F32, tag="rc2")
                    nc.vector.reciprocal_approx_fast(rc2[:], bc[:])
                    if i == 0:
                        nc.vector.tensor_mul(ot_tiles[jp][0:DH, :], asb[0:DH, :], rc2[:])
                    else:
                        # DVE can't shift partitions; stage then DMA into rows
                        # 64:128 (DMAs deferred to tq end, off the jp critical
                        # path of the shared DMA FIFO ring)
                        stg = p_rc.tile([DH, TB], BF16, tag="stg")
                        nc.vector.tensor_mul(stg[:], asb[0:DH, :], rc2[:])
                        stg_dmas.append((jp, stg))

            for jp_, stg_ in stg_dmas:
                nc.sync.dma_start(ot_tiles[jp_][DH : 2 * DH, :], stg_[:])
            stg_dmas = []
            pending = [(ot_tiles, tq, nb, ts) for nb in range(2) for ts in range(4)]

        for args in pending:
            emit_po_chain(*args)

    nc.compile()
    return nc


def kernel(**inputs: np.ndarray) -> np.ndarray:
    query = np.asarray(inputs["query"], dtype=np.float32)
    key = np.asarray(inputs["key"], dtype=np.float32)
    value = np.asarray(inputs["value"], dtype=np.float32)
    w_q = np.asarray(inputs["w_q"], dtype=np.float32)
    b_q = np.asarray(inputs["b_q"], dtype=np.float32)
    w_k = np.asarray(inputs["w_k"], dtype=np.float32)
    b_k = np.asarray(inputs["b_k"], dtype=np.float32)
    w_v = np.asarray(inputs["w_v"], dtype=np.float32)
    b_v = np.asarray(inputs["b_v"], dtype=np.float32)
    w_o = np.asarray(inputs["w_o"], dtype=np.float32)
    b_o = np.asarray(inputs["b_o"], dtype=np.float32)

    nc = build_kernel()

    in_maps = []
    for c in range(N_CORES):
        b = c // 2
        hh = c % 2
        sl = slice(hh * HALF, (hh + 1) * HALF)
        in_maps.append(
            {
                "xqT": np.ascontiguousarray(query[b].T).astype(NPBF16),
                "xkT": np.ascontiguousarray(key[b].T).astype(NPBF16),
                "xvT": np.ascontiguousarray(value[b].T).astype(NPBF16),
                "wqT": np.ascontiguousarray(w_q[sl, :].T).astype(NPBF16),
                "wkT": np.ascontiguousarray(w_k[sl, :].T).astype(NPBF16),
                "wvT": np.ascontiguousarray(w_v[sl, :].T).astype(NPBF16),
                "woT": np.ascontiguousarray(w_o[:, sl].T).astype(NPBF16),
                "bq": np.ascontiguousarray(b_q[sl].reshape(HALF, 1)),
                "bk": np.ascontiguousarray(b_k[sl].reshape(HALF, 1)),
            }
        )

    res = run_bass_kernel_spmd(nc, in_maps, core_ids=list(range(N_CORES)))

    const_row = (b_v[None, :] @ w_o.T + b_o[None, :]).astype(np.float32)
    out = np.empty((B, T, D), dtype=np.float32)
    for b in range(B):
        out[b] = res.results[2 * b]["partial"] + res.results[2 * b + 1]["partial"]
        out[b] += const_row
    return out
